# revision 15
# baseline (speedup 1.0000x reference)
"""GTCN kernel on 8 Trainium2 NeuronCores.

Strategy (per sharding_hint): data-parallel over batch B=64 across the 8
cores (8 samples each); all params replicated. The whole network runs as a
single XLA program per core via jax.jit + shard_map on the axon-tunneled
NeuronCores. Exact train-mode BatchNorm is kept by all-reducing per-timestep
(GCN layers) / per-channel (conv layers) sum and sum-of-squares statistics
across cores with lax.psum before normalizing.

To keep the TensorEngine on large dense GEMMs (the naive per-edge einsum
lowers to thousands of tiny K=25 matmuls), the graph is restructured into a
fixed (b*t, channels) layout:
  - GCN layer L: self-loop linear + edge-message scatter-add are folded into
    one dense weight  Wfull[(s,c),(d,f)] = delta_{sd}*sW[c,f] + A[d,s]*W[c,f]
    (A = (V,V) edge-count adjacency, duplicates accumulate as in the
    reference's .at[].add). One GEMM per layer: (b*t, V*Cin) @ (V*Cin, V*Hd).
  - Temporal convs: tap-unrolled into 3 shifted GEMMs over the padded time
    axis, staying in (b, t, chan) layout (no transpose to NCH).
GEMM operands are cast to bf16 (fp32 accumulate); BN statistics and
normalization stay fp32. Tolerance budget is 2e-2 overall.

Shapes hardcoded per spec: B=64, C=3, T=512, V=25, Hd=64, E=48, NC=60.
"""

import numpy as np

BN_EPS = 1e-5
_B, _C, _T, _V, _HD, _NC = 64, 3, 512, 25, 64, 60
_NCORES = 8

_cache = {}


def _build():
    if "fn" in _cache:
        return _cache["fn"]

    import jax
    import jax.numpy as jnp
    from jax import lax
    from jax.sharding import Mesh, PartitionSpec as P

    try:
        from jax.experimental.shard_map import shard_map
    except ImportError:  # newer jax
        from jax.experimental import shard_map as _sm
        shard_map = _sm.shard_map

    try:
        devs = jax.devices("axon")
    except RuntimeError:
        devs = jax.devices()
    devs = devs[:_NCORES]
    mesh = Mesh(np.asarray(devs), ("b",))
    bf16 = jnp.bfloat16
    f32 = jnp.float32

    def _mm(a, w):
        return jnp.matmul(a.astype(bf16), w, preferred_element_type=f32)

    def _bn_relu(H, gamma, beta):
        Hr = H.reshape(-1, _T, _V, _HD)
        s = lax.psum(Hr.sum(axis=(0, 2)), "b")          # (T, HD)
        sq = lax.psum((Hr * Hr).sum(axis=(0, 2)), "b")  # (T, HD)
        denom = float(_B * _V)
        mean = s / denom
        var = sq / denom - mean * mean
        scale = gamma * lax.rsqrt(var + BN_EPS)         # (T, HD)
        shift = beta - mean * scale
        Hn = jax.nn.relu(Hr * scale[None, :, None, :] + shift[None, :, None, :])
        return Hn.reshape(-1, _V * _HD).astype(bf16)    # (b*T, V*HD)

    def _gcn_bn_relu(Z, Wfull, bias, gamma, beta):
        # Z: (b*T, V*Cin) bf16 -> H_pre: (b*T, V*HD), per-(t, f) BN over (B, V)
        H = _mm(Z, Wfull) + bias[None, :]
        return _bn_relu(H, gamma, beta)

    def _gcn2_bn_relu(H1, W2cat, A_bf, bias, gamma, beta):
        # Factored layer 2: per-node transform as one fat GEMM, then the
        # (V,V) adjacency mix as a thin-K GEMM with forced (V, bt*HD)
        # orientation (K=M=25, huge N) instead of a dense 1600x1600 weight.
        # H1: (b*T, V*HD) bf16.
        SM = _mm(H1.reshape(-1, _HD), W2cat)            # (b*T*V, 2*HD) f32
        SM = SM.reshape(-1, _V, 2 * _HD)
        S = SM[:, :, :_HD]                              # (b*T, V, HD) self
        M2 = SM[:, :, _HD:]                             # (b*T, V, HD) messages
        mixed = jnp.einsum("ds,bsf->bdf", A_bf, M2.astype(bf16),
                           preferred_element_type=f32)  # (bt, V, HD)
        H = (S + mixed).reshape(-1, _V * _HD) + bias[None, :]
        return _bn_relu(H, gamma, beta)

    def _conv_bn_relu(Z, taps, bias, gamma, beta, dilation):
        # Z: (b, T, Cin) bf16; taps: (3, Cin, O) bf16; 'same' conv along T,
        # then per-channel BN over (B, T).
        pad = dilation
        Zp = jnp.pad(Z, ((0, 0), (pad, pad), (0, 0)))
        y = _mm(Zp[:, 0:_T, :], taps[0])
        y = y + _mm(Zp[:, dilation:dilation + _T, :], taps[1])
        y = y + _mm(Zp[:, 2 * dilation:2 * dilation + _T, :], taps[2])
        y = y + bias[None, None, :]                     # (b, T, O) f32
        s = lax.psum(y.sum(axis=(0, 1)), "b")           # (O,)
        sq = lax.psum((y * y).sum(axis=(0, 1)), "b")
        denom = float(_B * _T)
        mean = s / denom
        var = sq / denom - mean * mean
        scale = gamma * lax.rsqrt(var + BN_EPS)
        shift = beta - mean * scale
        yn = jax.nn.relu(y * scale[None, None, :] + shift[None, None, :])
        return yn.astype(bf16)

    def _fwd(X, Wfull1, bias1, g1, b1, W2cat, A_bf, bias2, g2, b2,
             taps1, c1b, tg1, tb1, taps2, c2b, tg2, tb2, fcW, fcb):
        # X: (b_local, C, T, V)
        bl = X.shape[0]
        Xt = jnp.transpose(X, (0, 2, 3, 1))             # (b, T, V, C)
        Z = Xt.reshape(bl * _T, _V * _C).astype(bf16)
        H = _gcn_bn_relu(Z, Wfull1, bias1, g1, b1)      # (b*T, V*HD)
        H = _gcn2_bn_relu(H, W2cat, A_bf, bias2, g2, b2)  # (b*T, V*HD)
        H = H.reshape(bl, _T, _V * _HD)
        y = _conv_bn_relu(H, taps1, c1b, tg1, tb1, dilation=1)   # (b, T, 64)
        y = _conv_bn_relu(y, taps2, c2b, tg2, tb2, dilation=2)   # (b, T, 128)
        z = y.astype(f32).mean(axis=1)                  # (b, 128)
        return z @ fcW + fcb                            # (b, NC)

    n_rep = 19
    fn = jax.jit(shard_map(
        _fwd, mesh=mesh,
        in_specs=(P("b"),) + (P(),) * n_rep,
        out_specs=P("b"),
        check_rep=False,
    ))
    _cache["fn"] = fn
    return fn


def _prep(edge_index, W1, s1W, s1b, W2, s2W, s2b, c1W, c2W):
    """Host-side weight folding (data-independent, tiny)."""
    f32 = np.float32
    A = np.zeros((_V, _V), dtype=f32)
    np.add.at(A, (edge_index[:, 1], edge_index[:, 0]), 1.0)

    # Wfull[(s,c),(d,f)] = delta_{sd} sW[c,f] + A[d,s] W[c,f]
    Wfull1 = (np.kron(np.eye(_V, dtype=f32), s1W.astype(f32))
              + np.einsum("ds,cf->scdf", A, W1.astype(f32)).reshape(
                  _V * _C, _V * _HD))
    W2cat = np.concatenate([s2W.astype(f32), W2.astype(f32)], axis=1)  # (HD, 2HD)
    bias1 = np.tile(s1b.astype(f32), _V)                # (V*HD,)
    bias2 = np.tile(s2b.astype(f32), _V)
    taps1 = np.ascontiguousarray(np.transpose(c1W.astype(f32), (2, 1, 0)))
    taps2 = np.ascontiguousarray(np.transpose(c2W.astype(f32), (2, 1, 0)))
    return Wfull1, bias1, W2cat, A, bias2, taps1, taps2


def kernel(X, edge_index, W1, s1W, s1b, g1, b1, W2, s2W, s2b, g2, b2,
           c1W, c1b, tg1, tb1, c2W, c2b, tg2, tb2, fcW, fcb):
    X = np.ascontiguousarray(np.asarray(X, dtype=np.float32))
    edge_index = np.asarray(edge_index)

    Wfull1, bias1, W2cat, A, bias2, taps1, taps2 = _prep(
        edge_index, W1, s1W, s1b, W2, s2W, s2b, c1W, c2W)

    import ml_dtypes
    f32 = np.float32
    bf16 = ml_dtypes.bfloat16
    args = [np.asarray(Wfull1, bf16), np.asarray(bias1, f32),
            np.asarray(g1, f32), np.asarray(b1, f32),
            np.asarray(W2cat, bf16), np.asarray(A, bf16),
            np.asarray(bias2, f32),
            np.asarray(g2, f32), np.asarray(b2, f32),
            np.asarray(taps1, bf16), np.asarray(c1b, f32),
            np.asarray(tg1, f32), np.asarray(tb1, f32),
            np.asarray(taps2, bf16), np.asarray(c2b, f32),
            np.asarray(tg2, f32), np.asarray(tb2, f32),
            np.asarray(fcW, f32), np.asarray(fcb, f32)]

    fn = _build()
    out = fn(X, *args)
    return np.asarray(out, dtype=np.float32)


# revision 16
# speedup vs baseline: 1.1961x; 1.1961x over previous
"""GTCN kernel on 8 Trainium2 NeuronCores.

Strategy (per sharding_hint): data-parallel over batch B=64 across the 8
cores (8 samples each); all params replicated. The whole network runs as a
single XLA program per core via jax.jit + shard_map on the axon-tunneled
NeuronCores. Exact train-mode BatchNorm is kept by all-reducing per-timestep
(GCN layers) / per-channel (conv layers) sum and sum-of-squares statistics
across cores with lax.psum before normalizing.

To keep the TensorEngine on large dense GEMMs (the naive per-edge einsum
lowers to thousands of tiny K=25 matmuls), the graph is restructured into a
fixed (b*t, channels) layout:
  - GCN layer L: self-loop linear + edge-message scatter-add are folded into
    one dense weight  Wfull[(s,c),(d,f)] = delta_{sd}*sW[c,f] + A[d,s]*W[c,f]
    (A = (V,V) edge-count adjacency, duplicates accumulate as in the
    reference's .at[].add). One GEMM per layer: (b*t, V*Cin) @ (V*Cin, V*Hd).
  - Temporal convs: tap-unrolled into 3 shifted GEMMs over the padded time
    axis, staying in (b, t, chan) layout (no transpose to NCH).
GEMM operands are cast to bf16 (fp32 accumulate); BN statistics and
normalization stay fp32. Tolerance budget is 2e-2 overall.

Shapes hardcoded per spec: B=64, C=3, T=512, V=25, Hd=64, E=48, NC=60.
"""

import numpy as np

BN_EPS = 1e-5
_B, _C, _T, _V, _HD, _NC = 64, 3, 512, 25, 64, 60
_NCORES = 8

_cache = {}


def _build():
    if "fn" in _cache:
        return _cache["fn"]

    import jax
    import jax.numpy as jnp
    from jax import lax
    from jax.sharding import Mesh, PartitionSpec as P

    try:
        from jax.experimental.shard_map import shard_map
    except ImportError:  # newer jax
        from jax.experimental import shard_map as _sm
        shard_map = _sm.shard_map

    try:
        devs = jax.devices("axon")
    except RuntimeError:
        devs = jax.devices()
    devs = devs[:_NCORES]
    mesh = Mesh(np.asarray(devs), ("b",))
    bf16 = jnp.bfloat16
    f32 = jnp.float32

    def _mm(a, w):
        return jnp.matmul(a.astype(bf16), w, preferred_element_type=f32)

    def _bn_relu(H, gamma, beta):
        Hr = H.reshape(-1, _T, _V, _HD)
        s = lax.psum(Hr.sum(axis=(0, 2)), "b")          # (T, HD)
        sq = lax.psum((Hr * Hr).sum(axis=(0, 2)), "b")  # (T, HD)
        denom = float(_B * _V)
        mean = s / denom
        var = sq / denom - mean * mean
        scale = gamma * lax.rsqrt(var + BN_EPS)         # (T, HD)
        shift = beta - mean * scale
        Hn = jax.nn.relu(Hr * scale[None, :, None, :] + shift[None, :, None, :])
        return Hn.reshape(-1, _V * _HD).astype(bf16)    # (b*T, V*HD)

    def _gcn_bn_relu(Z, Wfull, bias, gamma, beta):
        # Z: (b*T, V*Cin) bf16 -> H_pre: (b*T, V*HD), per-(t, f) BN over (B, V)
        H = _mm(Z, Wfull) + bias[None, :]
        return _bn_relu(H, gamma, beta)

    def _gcn2_bn_relu(H1, W2cat, A_bf, bias, gamma, beta):
        # Factored layer 2: per-node transform as one fat GEMM, then the
        # (V,V) adjacency mix as a thin-K GEMM with forced (V, bt*HD)
        # orientation (K=M=25, huge N) instead of a dense 1600x1600 weight.
        # H1: (b*T, V*HD) bf16.
        SM = _mm(H1.reshape(-1, _HD), W2cat)            # (b*T*V, 2*HD) f32
        SM = SM.reshape(-1, _V, 2 * _HD)
        S = SM[:, :, :_HD]                              # (b*T, V, HD) self
        M2 = SM[:, :, _HD:]                             # (b*T, V, HD) messages
        M2t = M2.astype(bf16).transpose(1, 0, 2).reshape(_V, -1)  # (V, bt*HD)
        mixedT = jnp.matmul(A_bf, M2t, preferred_element_type=f32)  # (V, bt*HD)
        mixed = mixedT.reshape(_V, -1, _HD).transpose(1, 0, 2)    # (bt, V, HD)
        H = (S + mixed).reshape(-1, _V * _HD) + bias[None, :]
        return _bn_relu(H, gamma, beta)

    def _conv_bn_relu(Z, taps, bias, gamma, beta, dilation):
        # Z: (b, T, Cin) bf16; taps: (3, Cin, O) bf16; 'same' conv along T,
        # then per-channel BN over (B, T).
        pad = dilation
        Zp = jnp.pad(Z, ((0, 0), (pad, pad), (0, 0)))
        y = _mm(Zp[:, 0:_T, :], taps[0])
        y = y + _mm(Zp[:, dilation:dilation + _T, :], taps[1])
        y = y + _mm(Zp[:, 2 * dilation:2 * dilation + _T, :], taps[2])
        y = y + bias[None, None, :]                     # (b, T, O) f32
        s = lax.psum(y.sum(axis=(0, 1)), "b")           # (O,)
        sq = lax.psum((y * y).sum(axis=(0, 1)), "b")
        denom = float(_B * _T)
        mean = s / denom
        var = sq / denom - mean * mean
        scale = gamma * lax.rsqrt(var + BN_EPS)
        shift = beta - mean * scale
        yn = jax.nn.relu(y * scale[None, None, :] + shift[None, None, :])
        return yn.astype(bf16)

    def _fwd(X, Wfull1, bias1, g1, b1, W2cat, A_bf, bias2, g2, b2,
             taps1, c1b, tg1, tb1, taps2, c2b, tg2, tb2, fcW, fcb):
        # X: (b_local, C, T, V)
        bl = X.shape[0]
        Xt = jnp.transpose(X, (0, 2, 3, 1))             # (b, T, V, C)
        Z = Xt.reshape(bl * _T, _V * _C).astype(bf16)
        H = _gcn_bn_relu(Z, Wfull1, bias1, g1, b1)      # (b*T, V*HD)
        H = _gcn2_bn_relu(H, W2cat, A_bf, bias2, g2, b2)  # (b*T, V*HD)
        H = H.reshape(bl, _T, _V * _HD)
        y = _conv_bn_relu(H, taps1, c1b, tg1, tb1, dilation=1)   # (b, T, 64)
        y = _conv_bn_relu(y, taps2, c2b, tg2, tb2, dilation=2)   # (b, T, 128)
        z = y.astype(f32).mean(axis=1)                  # (b, 128)
        return z @ fcW + fcb                            # (b, NC)

    n_rep = 19
    fn = jax.jit(shard_map(
        _fwd, mesh=mesh,
        in_specs=(P("b"),) + (P(),) * n_rep,
        out_specs=P("b"),
        check_rep=False,
    ))
    _cache["fn"] = fn
    return fn


def _prep(edge_index, W1, s1W, s1b, W2, s2W, s2b, c1W, c2W):
    """Host-side weight folding (data-independent, tiny)."""
    f32 = np.float32
    A = np.zeros((_V, _V), dtype=f32)
    np.add.at(A, (edge_index[:, 1], edge_index[:, 0]), 1.0)

    # Wfull[(s,c),(d,f)] = delta_{sd} sW[c,f] + A[d,s] W[c,f]
    Wfull1 = (np.kron(np.eye(_V, dtype=f32), s1W.astype(f32))
              + np.einsum("ds,cf->scdf", A, W1.astype(f32)).reshape(
                  _V * _C, _V * _HD))
    W2cat = np.concatenate([s2W.astype(f32), W2.astype(f32)], axis=1)  # (HD, 2HD)
    bias1 = np.tile(s1b.astype(f32), _V)                # (V*HD,)
    bias2 = np.tile(s2b.astype(f32), _V)
    taps1 = np.ascontiguousarray(np.transpose(c1W.astype(f32), (2, 1, 0)))
    taps2 = np.ascontiguousarray(np.transpose(c2W.astype(f32), (2, 1, 0)))
    return Wfull1, bias1, W2cat, A, bias2, taps1, taps2


def kernel(X, edge_index, W1, s1W, s1b, g1, b1, W2, s2W, s2b, g2, b2,
           c1W, c1b, tg1, tb1, c2W, c2b, tg2, tb2, fcW, fcb):
    X = np.ascontiguousarray(np.asarray(X, dtype=np.float32))
    edge_index = np.asarray(edge_index)

    Wfull1, bias1, W2cat, A, bias2, taps1, taps2 = _prep(
        edge_index, W1, s1W, s1b, W2, s2W, s2b, c1W, c2W)

    import ml_dtypes
    f32 = np.float32
    bf16 = ml_dtypes.bfloat16
    args = [np.asarray(Wfull1, bf16), np.asarray(bias1, f32),
            np.asarray(g1, f32), np.asarray(b1, f32),
            np.asarray(W2cat, bf16), np.asarray(A, bf16),
            np.asarray(bias2, f32),
            np.asarray(g2, f32), np.asarray(b2, f32),
            np.asarray(taps1, bf16), np.asarray(c1b, f32),
            np.asarray(tg1, f32), np.asarray(tb1, f32),
            np.asarray(taps2, bf16), np.asarray(c2b, f32),
            np.asarray(tg2, f32), np.asarray(tb2, f32),
            np.asarray(fcW, f32), np.asarray(fcb, f32)]

    fn = _build()
    out = fn(X, *args)
    return np.asarray(out, dtype=np.float32)


# revision 18
# speedup vs baseline: 1.5340x; 1.2825x over previous
"""GTCN kernel on 8 Trainium2 NeuronCores.

Strategy (per sharding_hint): data-parallel over batch B=64 across the 8
cores (8 samples each); all params replicated. The whole network runs as a
single XLA program per core via jax.jit + shard_map on the axon-tunneled
NeuronCores. Exact train-mode BatchNorm is kept by all-reducing per-timestep
(GCN layers) / per-channel (conv layers) sum and sum-of-squares statistics
across cores with lax.psum before normalizing.

To keep the TensorEngine on large dense GEMMs (the naive per-edge einsum
lowers to thousands of tiny K=25 matmuls), the graph is restructured into a
fixed (b*t, channels) layout:
  - GCN layer L: self-loop linear + edge-message scatter-add are folded into
    one dense weight  Wfull[(s,c),(d,f)] = delta_{sd}*sW[c,f] + A[d,s]*W[c,f]
    (A = (V,V) edge-count adjacency, duplicates accumulate as in the
    reference's .at[].add). One GEMM per layer: (b*t, V*Cin) @ (V*Cin, V*Hd).
  - Temporal convs: tap-unrolled into 3 shifted GEMMs over the padded time
    axis, staying in (b, t, chan) layout (no transpose to NCH).
GEMM operands are cast to bf16 (fp32 accumulate); BN statistics and
normalization stay fp32. Tolerance budget is 2e-2 overall.

Shapes hardcoded per spec: B=64, C=3, T=512, V=25, Hd=64, E=48, NC=60.
"""

import numpy as np

BN_EPS = 1e-5
_B, _C, _T, _V, _HD, _NC = 64, 3, 512, 25, 64, 60
_NCORES = 8

_cache = {}


def _build():
    if "fn" in _cache:
        return _cache["fn"]

    import jax
    import jax.numpy as jnp
    from jax import lax
    from jax.sharding import Mesh, PartitionSpec as P

    try:
        from jax.experimental.shard_map import shard_map
    except ImportError:  # newer jax
        from jax.experimental import shard_map as _sm
        shard_map = _sm.shard_map

    try:
        devs = jax.devices("axon")
    except RuntimeError:
        devs = jax.devices()
    devs = devs[:_NCORES]
    mesh = Mesh(np.asarray(devs), ("b",))
    bf16 = jnp.bfloat16
    f32 = jnp.float32

    def _mm(a, w):
        return jnp.matmul(a.astype(bf16), w, preferred_element_type=f32)

    def _bn_relu(H, gamma, beta):
        Hr = H.reshape(-1, _T, _V, _HD)
        s = lax.psum(Hr.sum(axis=(0, 2)), "b")          # (T, HD)
        sq = lax.psum((Hr * Hr).sum(axis=(0, 2)), "b")  # (T, HD)
        denom = float(_B * _V)
        mean = s / denom
        var = sq / denom - mean * mean
        scale = gamma * lax.rsqrt(var + BN_EPS)         # (T, HD)
        shift = beta - mean * scale
        Hn = jax.nn.relu(Hr * scale[None, :, None, :] + shift[None, :, None, :])
        return Hn.reshape(-1, _V * _HD).astype(bf16)    # (b*T, V*HD)

    def _gcn_bn_relu(Z, Wfull, bias, gamma, beta):
        # Z: (b*T, V*Cin) bf16 -> H_pre: (b*T, V*HD), per-(t, f) BN over (B, V)
        H = _mm(Z, Wfull) + bias[None, :]
        return _bn_relu(H, gamma, beta)

    def _gcn2_bn_relu(H1, W2cat, A_bf, bias, gamma, beta):
        # Factored layer 2: per-node transform as one fat GEMM, then the
        # (V,V) adjacency mix as a thin-K GEMM with forced (V, bt*HD)
        # orientation (K=M=25, huge N) instead of a dense 1600x1600 weight.
        # H1: (b*T, V*HD) bf16.
        SM = _mm(H1.reshape(-1, _HD), W2cat)            # (b*T*V, 2*HD) f32
        SM = SM.reshape(-1, _V, 2 * _HD)
        S = SM[:, :, :_HD]                              # (b*T, V, HD) self
        M2 = SM[:, :, _HD:]                             # (b*T, V, HD) messages
        M2t = M2.astype(bf16).transpose(1, 0, 2).reshape(_V, -1)  # (V, bt*HD)
        mixedT = jnp.matmul(A_bf, M2t, preferred_element_type=f32)  # (V, bt*HD)
        mixed = mixedT.reshape(_V, -1, _HD).transpose(1, 0, 2)    # (bt, V, HD)
        H = (S + mixed).reshape(-1, _V * _HD) + bias[None, :]
        return _bn_relu(H, gamma, beta)

    def _conv_bn_relu(Z, Wcat, bias, gamma, beta, dilation):
        # Z: (b, T, Cin) bf16; Wcat: (Cin, 3*O) bf16 (taps stacked on the
        # output side). One fat GEMM over the padded sequence, then the tap
        # sum becomes three shifted slice-adds of the small output.
        pad = dilation
        O = Wcat.shape[1] // 3
        Zp = jnp.pad(Z, ((0, 0), (pad, pad), (0, 0)))   # (b, T+2p, Cin)
        U = _mm(Zp, Wcat)                               # (b, T+2p, 3O) f32
        y = (U[:, 0:_T, 0:O]
             + U[:, dilation:dilation + _T, O:2 * O]
             + U[:, 2 * dilation:2 * dilation + _T, 2 * O:3 * O])
        y = y + bias[None, None, :]                     # (b, T, O) f32
        s = lax.psum(y.sum(axis=(0, 1)), "b")           # (O,)
        sq = lax.psum((y * y).sum(axis=(0, 1)), "b")
        denom = float(_B * _T)
        mean = s / denom
        var = sq / denom - mean * mean
        scale = gamma * lax.rsqrt(var + BN_EPS)
        shift = beta - mean * scale
        yn = jax.nn.relu(y * scale[None, None, :] + shift[None, None, :])
        return yn.astype(bf16)

    def _fwd(X, Wfull1, bias1, g1, b1, W2cat, A_bf, bias2, g2, b2,
             taps1, c1b, tg1, tb1, taps2, c2b, tg2, tb2, fcW, fcb):
        # X: (b_local, C, T, V)
        bl = X.shape[0]
        Xt = jnp.transpose(X, (0, 2, 3, 1))             # (b, T, V, C)
        Z = Xt.reshape(bl * _T, _V * _C).astype(bf16)
        H = _gcn_bn_relu(Z, Wfull1, bias1, g1, b1)      # (b*T, V*HD)
        H = _gcn2_bn_relu(H, W2cat, A_bf, bias2, g2, b2)  # (b*T, V*HD)
        H = H.reshape(bl, _T, _V * _HD)
        y = _conv_bn_relu(H, taps1, c1b, tg1, tb1, dilation=1)   # (b, T, 64)
        y = _conv_bn_relu(y, taps2, c2b, tg2, tb2, dilation=2)   # (b, T, 128)
        z = y.astype(f32).mean(axis=1)                  # (b, 128)
        return z @ fcW + fcb                            # (b, NC)

    n_rep = 19
    fn = jax.jit(shard_map(
        _fwd, mesh=mesh,
        in_specs=(P("b"),) + (P(),) * n_rep,
        out_specs=P("b"),
        check_rep=False,
    ))
    _cache["fn"] = fn
    return fn


def _prep(edge_index, W1, s1W, s1b, W2, s2W, s2b, c1W, c2W):
    """Host-side weight folding (data-independent, tiny)."""
    f32 = np.float32
    A = np.zeros((_V, _V), dtype=f32)
    np.add.at(A, (edge_index[:, 1], edge_index[:, 0]), 1.0)

    # Wfull[(s,c),(d,f)] = delta_{sd} sW[c,f] + A[d,s] W[c,f]
    Wfull1 = (np.kron(np.eye(_V, dtype=f32), s1W.astype(f32))
              + np.einsum("ds,cf->scdf", A, W1.astype(f32)).reshape(
                  _V * _C, _V * _HD))
    W2cat = np.concatenate([s2W.astype(f32), W2.astype(f32)], axis=1)  # (HD, 2HD)
    bias1 = np.tile(s1b.astype(f32), _V)                # (V*HD,)
    bias2 = np.tile(s2b.astype(f32), _V)
    # (O, Cin, 3) -> (Cin, 3*O): taps stacked along the output dim
    taps1 = np.ascontiguousarray(
        np.transpose(c1W.astype(f32), (1, 2, 0)).reshape(c1W.shape[1], -1))
    taps2 = np.ascontiguousarray(
        np.transpose(c2W.astype(f32), (1, 2, 0)).reshape(c2W.shape[1], -1))
    return Wfull1, bias1, W2cat, A, bias2, taps1, taps2


def kernel(X, edge_index, W1, s1W, s1b, g1, b1, W2, s2W, s2b, g2, b2,
           c1W, c1b, tg1, tb1, c2W, c2b, tg2, tb2, fcW, fcb):
    X = np.ascontiguousarray(np.asarray(X, dtype=np.float32))
    edge_index = np.asarray(edge_index)

    Wfull1, bias1, W2cat, A, bias2, taps1, taps2 = _prep(
        edge_index, W1, s1W, s1b, W2, s2W, s2b, c1W, c2W)

    import ml_dtypes
    f32 = np.float32
    bf16 = ml_dtypes.bfloat16
    args = [np.asarray(Wfull1, bf16), np.asarray(bias1, f32),
            np.asarray(g1, f32), np.asarray(b1, f32),
            np.asarray(W2cat, bf16), np.asarray(A, bf16),
            np.asarray(bias2, f32),
            np.asarray(g2, f32), np.asarray(b2, f32),
            np.asarray(taps1, bf16), np.asarray(c1b, f32),
            np.asarray(tg1, f32), np.asarray(tb1, f32),
            np.asarray(taps2, bf16), np.asarray(c2b, f32),
            np.asarray(tg2, f32), np.asarray(tb2, f32),
            np.asarray(fcW, f32), np.asarray(fcb, f32)]

    fn = _build()
    out = fn(X, *args)
    return np.asarray(out, dtype=np.float32)
